# revision 25
# baseline (speedup 1.0000x reference)
"""Trainium2 Bass kernel for nn_ArcticDecoderLayer (8-core SPMD).

Sharding:
  - Attention + parallel-residual MLP: token-parallel (core c owns tokens
    [256c, 256c+256)). K/V are computed locally (with RoPE) and AllGathered.
  - MoE: expert-parallel, computed densely exactly like the reference
    (every expert sees every token, outputs weighted by top-2 routes).
    Core c holds expert c; partial outputs are ReduceScattered back to the
    token owner (split in two feature halves to overlap with compute).

Layout convention: activations are stored feature-major ("transposed",
[features on partitions, tokens on free axis]) so every matmul contracts
over the partition axis. RMS-norm reductions over features use a
matmul-with-ones trick; per-token scales are applied via
gpsimd.partition_broadcast + DVE multiplies.

Precision: matmuls in bf16 except the gate (fp32, so top-2 routing matches
the reference) and the expert FFN (fp8 e4m3 with DoubleRow; weights are
pre-scaled on the host -- g rows x64, u rows x8, w2 x64 -- and the 1/512
descale is folded into the route row).

Host-side preprocessing folds input_ln_w / residual_ln_w / post_ln_w and the
attention 1/sqrt(hd) scale into the consumer weight matrices, precomputes
RoPE cos/sin tables from `positions`, and builds the additive causal mask.
"""

import numpy as np
import ml_dtypes

import concourse.bass as bass
import concourse.mybir as mybir
import concourse.tile as tile
from concourse import bacc
from concourse.bass_utils import run_bass_kernel_spmd
from concourse.masks import make_identity

# Problem constants (hardcoded per contract).
T, H, NH, NKV, HD = 2048, 1024, 16, 4, 64
I, E = 2048, 8
EPS = 1e-5
ROPE_BASE = 10000.0
NCORES = 8
TPC = T // NCORES          # 256 tokens per core
QKF = (NH + NKV) * HD      # 1280 q+k features
VF = NKV * HD              # 256 v features
HALF = HD // 2             # 32
NEG = -1.0e9
# fp8 pre-scales: g_psum = SG*g (silu applies 1/SG); act = silu(g)*SU*u;
# eo_psum = SW2*SU*eo_true -> descale 1/(SU*SW2) folded into the route row.
SG, SU, SW2 = 64.0, 8.0, 64.0
DESCALE = 1.0 / (SU * SW2)

BF = mybir.dt.bfloat16
F32 = mybir.dt.float32
F8 = mybir.dt.float8e4

bf16 = ml_dtypes.bfloat16


def _r3(ap):
    """[K, M] dram AP -> [128, K//128, M] partition-tiled view."""
    return ap.rearrange("(o p) m -> p o m", p=128)


def build_nc(reps=1, stages=99):
    nc = bacc.Bacc("TRN2", target_bir_lowering=False, debug=False,
                   num_devices=NCORES)

    # ---- per-core external inputs ----
    xT = nc.dram_tensor("xT", [128, 8, TPC], F32, kind="ExternalInput")
    cosr = nc.dram_tensor("cosr", [128, TPC], F32, kind="ExternalInput")
    sinr = nc.dram_tensor("sinr", [128, TPC], F32, kind="ExternalInput")
    maskT = nc.dram_tensor("maskT", [128, 16, TPC], BF, kind="ExternalInput")
    wqkT = nc.dram_tensor("wqkT", [10, 128, 8, 128], BF, kind="ExternalInput")
    wvT = nc.dram_tensor("wvT", [128, 8, VF], BF, kind="ExternalInput")
    woT = nc.dram_tensor("woT", [8, 128, 8, 128], BF, kind="ExternalInput")
    w13T = nc.dram_tensor("w13T", [16, 128, 8, 128], BF, kind="ExternalInput")
    w2rT = nc.dram_tensor("w2rT", [8, 128, 8, 128], BF, kind="ExternalInput")
    gT = nc.dram_tensor("gT", [H, E], F32, kind="ExternalInput")
    wsT = nc.dram_tensor("wsT", [32, 128, 8, 128], F8, kind="ExternalInput")
    w2sT = nc.dram_tensor("w2sT", [8, 128, 16, 128], F8, kind="ExternalInput")
    onehot = nc.dram_tensor("onehot", [E, 1], BF, kind="ExternalInput")
    yT = nc.dram_tensor("yT", [H, TPC], F32, kind="ExternalOutput")

    AX = mybir.AxisListType.X
    MUL = mybir.AluOpType.mult
    ADD = mybir.AluOpType.add
    SUB = mybir.AluOpType.subtract
    DR = mybir.MatmulPerfMode.DoubleRow

    with tile.TileContext(nc) as tc:
        with (
            tc.tile_pool(name="dram", bufs=1, space="DRAM") as dram,
            tc.tile_pool(name="const", bufs=1) as const,
            tc.tile_pool(name="persist", bufs=1) as persist,
        ):
            ones_col = const.tile([128, 1], F32, tag="ones")
            nc.vector.memset(ones_col[:], 1.0)
            eps_sb = const.tile([128, 1], F32, tag="eps")
            nc.vector.memset(eps_sb[:], EPS)
            ident = const.tile([128, 128], F32, tag="ident")
            make_identity(nc, ident[:])

            cos_sb = persist.tile([128, TPC], F32, tag="cos_sb")
            sin_sb = persist.tile([128, TPC], F32, tag="sin_sb")
            nc.sync.dma_start(cos_sb[:], cosr[:])
            nc.sync.dma_start(sin_sb[:], sinr[:])

            def emit_body(x_src, y_dst):
                # DRAM scratch for collectives (tags shared across reps ->
                # slot reuse serializes reps, which is what we want)
                kv_in = dram.tile([2, VF, TPC], BF, tag="kv_in")
                kv_all = dram.tile([NCORES, 2, VF, TPC], BF, tag="kv_all",
                                   addr_space="Shared")
                nag = dram.tile([H, TPC], F8, tag="nag")
                ag_all = dram.tile([NCORES, H, TPC], F8, tag="ag_all",
                                   addr_space="Shared")
                rag = dram.tile([E, TPC], BF, tag="rag")
                rag_all = dram.tile([NCORES, E, TPC], BF, tag="rag_all",
                                    addr_space="Shared")
                rs_in = [dram.tile([NCORES, H // 2, TPC], BF, tag=f"rs_in{i}",
                                   name=f"rs_in{i}")
                         for i in range(2)]
                rs_out = [dram.tile([H // 2, TPC], BF, tag=f"rs_out{i}",
                                    name=f"rs_out{i}")
                          for i in range(2)]

                # persistent sbuf (per body)
                x_sb = persist.tile([128, 8, TPC], F32, tag="x_sb")
                xn_sb = persist.tile([128, 8, TPC], BF, tag="xn_sb")
                resid_sb = persist.tile([128, 8, TPC], F32, tag="resid_sb")
                n_sb = persist.tile([128, 8, TPC], BF, tag="n_sb")
                resid2_sb = persist.tile([128, 8, TPC], F32, tag="resid2_sb")

                nc.sync.dma_start(x_sb[:], x_src)

                # ---------- helpers ----------
                def rms_scale(src_tiles, out_tiles, tmp_pool, psum_pool,
                              inv_out=None):
                    """out = src * rsqrt(mean_f(src^2)+eps), feature-major."""
                    ps = psum_pool.tile([1, TPC], F32, tag="ssq")
                    for o in range(8):
                        sq = tmp_pool.tile([128, TPC], F32, tag="sq")
                        nc.vector.tensor_tensor(sq[:], src_tiles[:, o, :],
                                                src_tiles[:, o, :], MUL)
                        nc.tensor.matmul(ps[:], ones_col[:], sq[:],
                                         start=(o == 0), stop=(o == 7))
                    sq2 = tmp_pool.tile([1, TPC], F32, tag="sqv")
                    nc.scalar.activation(sq2[:], ps[:],
                                         mybir.ActivationFunctionType.Sqrt,
                                         bias=eps_sb[0:1, :], scale=1.0 / H)
                    inv = (inv_out if inv_out is not None
                           else tmp_pool.tile([1, TPC], F32, tag="inv"))
                    nc.vector.reciprocal(inv[0:1, :], sq2[:])
                    invb = tmp_pool.tile([128, TPC], F32, tag="invb")
                    nc.gpsimd.partition_broadcast(invb[:], inv[0:1, :])
                    for o in range(8):
                        nc.vector.tensor_tensor(out_tiles[:, o, :],
                                                src_tiles[:, o, :], invb[:],
                                                MUL)

                def rope_tile(psum_ap, outs, tmp_pool):
                    """NeoX rope on a [128, TPC] psum tile holding 2 heads.

                    outs = (apA, apB): [64, TPC] destinations at partition
                    base 0 for heads at psum partitions [0:64] / [64:128]."""
                    for hh, out_ap in enumerate(outs):
                        b = 64 * hh
                        x1 = psum_ap[b:b + 32, :]
                        x2 = psum_ap[b + 32:b + 64, :]
                        ta = tmp_pool.tile([64, TPC], F32, tag="rope_a")
                        tb = tmp_pool.tile([64, TPC], F32, tag="rope_b")
                        nc.vector.tensor_tensor(ta[0:32, :], x1,
                                                cos_sb[b:b + 32, :], MUL)
                        nc.vector.tensor_tensor(tb[0:32, :], x2,
                                                sin_sb[b:b + 32, :], MUL)
                        nc.vector.tensor_tensor(out_ap[0:32, :],
                                                ta[0:32, :], tb[0:32, :], SUB)
                        nc.vector.tensor_tensor(ta[32:64, :], x2,
                                                cos_sb[b + 32:b + 64, :], MUL)
                        nc.vector.tensor_tensor(tb[32:64, :], x1,
                                                sin_sb[b + 32:b + 64, :], MUL)
                        nc.vector.tensor_tensor(out_ap[32:64, :],
                                                ta[32:64, :], tb[32:64, :],
                                                ADD)

                # ---------- stage 0: input rms norm ----------
                with (
                    tc.tile_pool(name="s0tmp", bufs=3) as s0tmp,
                    tc.tile_pool(name="s0ps", bufs=1, space="PSUM") as s0ps,
                ):
                    rms_scale(x_sb, xn_sb, s0tmp, s0ps)

                if stages < 1:
                    return
                # ---------- stage 1: qkv + rope + kv allgather ----------
                with (
                    tc.tile_pool(name="s1w", bufs=3) as s1w,
                    tc.tile_pool(name="s1ps", bufs=3, space="PSUM") as s1ps,
                    tc.tile_pool(name="s1tmp", bufs=2) as s1tmp,
                ):
                    q_sb = persist.tile([64, 16, TPC], BF, tag="q_sb")

                    def qk_tile(mt):
                        wt = s1w.tile([128, 8, 128], BF, tag="wqk", name="wt")
                        nc.sync.dma_start(wt[:], wqkT[mt])
                        ps = s1ps.tile([128, TPC], F32, tag="qk", name="ps")
                        for kt in range(8):
                            nc.tensor.matmul(ps[:], wt[:, kt, :],
                                             xn_sb[:, kt, :],
                                             start=(kt == 0), stop=(kt == 7))
                        if mt < 8:
                            rope_tile(ps[:], (q_sb[:, 2 * mt, :],
                                              q_sb[:, 2 * mt + 1, :]), s1tmp)
                        else:
                            kr = s1tmp.tile([64, 2, TPC], BF, tag="krope",
                                            name="kr")
                            rope_tile(ps[:], (kr[:, 0, :], kr[:, 1, :]), s1tmp)
                            nc.sync.dma_start(
                                kv_in[0, (mt - 8) * 128:(mt - 7) * 128, :]
                                .rearrange("(hh d) t -> d hh t", hh=2), kr[:])

                    # k and v first so the AllGather overlaps the q tiles
                    qk_tile(8)
                    qk_tile(9)
                    wvt = s1w.tile([128, 8, VF], BF, tag="wv")
                    nc.sync.dma_start(wvt[:], wvT[:])
                    for qt in range(2):
                        ps = s1ps.tile([128, VF], F32, tag="v", bufs=2)
                        for kt in range(8):
                            nc.tensor.matmul(
                                ps[:], xn_sb[:, kt, qt * 128:(qt + 1) * 128],
                                wvt[:, kt, :], start=(kt == 0), stop=(kt == 7))
                        vv = s1tmp.tile([128, VF], BF, tag="vcast")
                        nc.vector.tensor_copy(vv[:], ps[:])
                        nc.sync.dma_start(kv_in[1, qt * 128:(qt + 1) * 128, :],
                                          vv[:])
                    nc.gpsimd.collective_compute(
                        "AllGather", mybir.AluOpType.bypass,
                        replica_groups=[list(range(NCORES))],
                        ins=[kv_in[:].opt()], outs=[kv_all[:].opt()],
                    )
                    for mt in range(8):
                        qk_tile(mt)

                if stages < 2:
                    return
                # ---------- stage 2: attention ----------
                with (
                    tc.tile_pool(name="s2kv", bufs=1) as s2kv,
                    tc.tile_pool(name="s2probs", bufs=2) as s2probs,
                    tc.tile_pool(name="s2ps", bufs=2, space="PSUM") as s2ps,
                    tc.tile_pool(name="s2pa", bufs=3, space="PSUM") as s2pa,
                    tc.tile_pool(name="s2tmp", bufs=3) as s2tmp,
                ):
                    attn_sb = persist.tile([128, 8, TPC], BF, tag="attn_sb")
                    # K feature-major at base 0: [64 d, kv head, tok]
                    k_sb = s2kv.tile([64, NKV, T], BF, tag="k_sb")
                    for kh in range(NKV):
                        nc.sync.dma_start(
                            k_sb[:, kh, :]
                            .rearrange("d (b t) -> d b t", b=NCORES),
                            kv_all[:, 0, kh * 64:(kh + 1) * 64, :]
                            .rearrange("b d t -> d b t"))
                    # V token-major + ones column: [128 tok, kt, kh, 65]
                    v_sb2 = s2kv.tile([128, 16, NKV, 65], BF, tag="v_sb2")
                    for kt in range(16):
                        b, rr = kt // 2, (kt % 2) * 128
                        nc.sync.dma_start(
                            v_sb2[:, kt, :, 0:64],
                            kv_all[b, 1, rr:rr + 128, :]
                            .rearrange("t (kh d) -> t kh d", d=64))
                    nc.vector.memset(v_sb2[:, :, :, 64:65], 1.0)
                    # mask
                    mask_sb = s2kv.tile([128, 16, TPC], BF, tag="mask_sb")
                    nc.sync.dma_start(mask_sb[:], maskT[:])

                    for h in range(NH):
                        kh = h // NKV
                        q_rhs = q_sb[:, h, :]
                        probs = s2probs.tile([128, 16, TPC], BF, tag="probs")
                        for ktg in range(4):
                            ps = s2ps.tile([128, 4, TPC], F32, tag="sc")
                            for j in range(4):
                                kt = 4 * ktg + j
                                nc.tensor.matmul(
                                    ps[:, j, :],
                                    k_sb[:, kh, kt * 128:(kt + 1) * 128],
                                    q_rhs, start=True, stop=True)
                            nc.scalar.activation(
                                probs[:, 4 * ktg:4 * ktg + 4, :], ps[:],
                                mybir.ActivationFunctionType.Exp)
                            # causal mask is multiplicative 0/1 in bf16:
                            # exp(s + m) == exp(s) * m01
                            nc.vector.tensor_tensor(
                                probs[:, 4 * ktg:4 * ktg + 4, :],
                                probs[:, 4 * ktg:4 * ktg + 4, :],
                                mask_sb[:, 4 * ktg:4 * ktg + 4, :], MUL)
                        pa = s2pa.tile([128, TPC], F32, tag="pattn")
                        for kt in range(16):
                            nc.tensor.matmul(pa[0:65, :], v_sb2[:, kt, kh, :],
                                             probs[:, kt, :],
                                             start=(kt == 0), stop=(kt == 15))
                        rec = s2tmp.tile([1, TPC], F32, tag="rec")
                        nc.vector.reciprocal(rec[:], pa[64:65, :])
                        recb = s2tmp.tile([64, TPC], F32, tag="recb")
                        nc.gpsimd.partition_broadcast(recb[:], rec[0:1, :])
                        nc.vector.tensor_tensor(
                            attn_sb[(h % 2) * 64:(h % 2) * 64 + 64, h // 2, :],
                            pa[0:64, :], recb[:], MUL)

                if stages < 3:
                    return
                # ---------- stage 3: o-proj + residual ----------
                with (
                    tc.tile_pool(name="s3w", bufs=3) as s3w,
                    tc.tile_pool(name="s3ps", bufs=4, space="PSUM") as s3ps,
                ):
                    for mt in range(8):
                        wt = s3w.tile([128, 8, 128], BF, tag="wo")
                        nc.sync.dma_start(wt[:], woT[mt])
                        ps = s3ps.tile([128, TPC], F32, tag="o")
                        for kt in range(8):
                            nc.tensor.matmul(ps[:], wt[:, kt, :],
                                             attn_sb[:, kt, :],
                                             start=(kt == 0), stop=(kt == 7))
                        nc.vector.tensor_tensor(resid_sb[:, mt, :], ps[:],
                                                x_sb[:, mt, :], ADD)

                if stages < 4:
                    return
                # ------- stage 4: post norm + fp32 gate + routes + AG -------
                with (
                    tc.tile_pool(name="s4tmp", bufs=3) as s4tmp,
                    tc.tile_pool(name="s4ps", bufs=2, space="PSUM") as s4ps,
                ):
                    inv2 = s4tmp.tile([1, TPC], F32, tag="inv2", bufs=1)
                    rms_scale(resid_sb, n_sb, s4tmp, s4ps, inv_out=inv2)
                    nf8 = s4tmp.tile([128, 8, TPC], F8, tag="nf8", bufs=1)
                    for o in range(8):
                        nc.vector.tensor_copy(nf8[:, o, :], n_sb[:, o, :])
                    nc.sync.dma_start(
                        nag[:].rearrange("(o p) t -> p o t", p=128), nf8[:])
                    nc.gpsimd.collective_compute(
                        "AllGather", mybir.AluOpType.bypass,
                        replica_groups=[list(range(NCORES))],
                        ins=[nag[:].opt()], outs=[ag_all[:].opt()],
                    )
                    # inv_rms token-major: [TPC(2x128), 1] via matmul transpose
                    invt = s4tmp.tile([128, 2, 1], F32, tag="invt", bufs=1)
                    for qt in range(2):
                        pst = s4ps.tile([128, 1], F32, tag="invtp")
                        nc.tensor.matmul(pst[:],
                                         inv2[0:1, qt * 128:(qt + 1) * 128],
                                         ones_col[0:1, :], start=True,
                                         stop=True)
                        nc.vector.tensor_copy(invt[:, qt, :], pst[:])
                    # gate logits fp32 token-major: [128 tok, E]
                    gt_sb = s4tmp.tile([128, 8, E], F32, tag="gt", bufs=1)
                    nc.sync.dma_start(gt_sb[:], _r3(gT[:]))
                    for qt in range(2):
                        lg = s4ps.tile([128, E], F32, tag="lg")
                        for kt in range(8):
                            nc.tensor.matmul(
                                lg[:],
                                resid_sb[:, kt, qt * 128:(qt + 1) * 128],
                                gt_sb[:, kt, :], start=(kt == 0),
                                stop=(kt == 7))
                        lgs = s4tmp.tile([128, E], F32, tag="lgs")
                        nc.scalar.mul(lgs[:], lg[:], invt[:, qt, :])
                        # top-2 softmax renorm, token-major
                        m1 = s4tmp.tile([128, 1], F32, tag="m1")
                        nc.vector.reduce_max(m1[:], lgs[:], axis=AX)
                        negm = s4tmp.tile([128, 1], F32, tag="negm")
                        nc.vector.tensor_scalar_mul(negm[:], m1[:], -1.0)
                        ex = s4tmp.tile([128, E], F32, tag="ex")
                        nc.scalar.activation(ex[:], lgs[:],
                                             mybir.ActivationFunctionType.Exp,
                                             bias=negm[:])
                        msk = s4tmp.tile([128, E], F32, tag="msk")
                        nc.vector.tensor_tensor(msk[:], lgs[:],
                                                m1[:].to_broadcast([128, E]),
                                                mybir.AluOpType.is_ge)
                        nc.vector.tensor_scalar_mul(msk[:], msk[:], NEG)
                        nc.vector.tensor_tensor(msk[:], lgs[:], msk[:], ADD)
                        m2 = s4tmp.tile([128, 1], F32, tag="m2")
                        nc.vector.reduce_max(m2[:], msk[:], axis=AX)
                        keep = s4tmp.tile([128, E], F32, tag="keep")
                        nc.vector.tensor_tensor(keep[:], lgs[:],
                                                m2[:].to_broadcast([128, E]),
                                                mybir.AluOpType.is_ge)
                        nc.vector.tensor_tensor(keep[:], keep[:], ex[:], MUL)
                        den = s4tmp.tile([128, 1], F32, tag="den")
                        nc.vector.reduce_sum(den[:], keep[:], axis=AX)
                        rden = s4tmp.tile([128, 1], F32, tag="rden")
                        nc.vector.reciprocal(rden[:], den[:])
                        routes = s4tmp.tile([128, E], F32, tag="routes")
                        nc.scalar.mul(routes[:], keep[:], rden[:])
                        # transpose to [E, 128] and ship bf16
                        pt = s4ps.tile([128, 128], F32, tag="rt")
                        nc.tensor.transpose(pt[0:E, :], routes[:], ident[:])
                        rbf = s4tmp.tile([E, 128], BF, tag="rbf")
                        nc.vector.tensor_copy(rbf[:], pt[0:E, :])
                        nc.sync.dma_start(rag[:, qt * 128:(qt + 1) * 128],
                                          rbf[:])

                nc.gpsimd.collective_compute(
                    "AllGather", mybir.AluOpType.bypass,
                    replica_groups=[list(range(NCORES))],
                    ins=[rag[:].opt()], outs=[rag_all[:].opt()],
                )

                if stages < 5:
                    return
                # -------- stage 5: parallel residual MLP (local tokens) -----
                with (
                    tc.tile_pool(name="s5w", bufs=3) as s5w,
                    tc.tile_pool(name="s5ps", bufs=2, space="PSUM") as s5ps,
                    tc.tile_pool(name="s5act", bufs=1) as s5act,
                ):
                    act5 = s5act.tile([128, 8, TPC], BF, tag="act5")
                    for it in range(8):
                        wg = s5w.tile([128, 8, 128], BF, tag="w13g")
                        nc.sync.dma_start(wg[:], w13T[it])
                        wu = s5w.tile([128, 8, 128], BF, tag="w13u")
                        nc.sync.dma_start(wu[:], w13T[8 + it])
                        pg = s5ps.tile([128, TPC], F32, tag="pg")
                        for kt in range(8):
                            nc.tensor.matmul(pg[:], wg[:, kt, :],
                                             n_sb[:, kt, :],
                                             start=(kt == 0), stop=(kt == 7))
                        pu = s5ps.tile([128, TPC], F32, tag="pu")
                        for kt in range(8):
                            nc.tensor.matmul(pu[:], wu[:, kt, :],
                                             n_sb[:, kt, :],
                                             start=(kt == 0), stop=(kt == 7))
                        gs = s5w.tile([128, TPC], BF, tag="gsil")
                        nc.scalar.activation(gs[:], pg[:],
                                             mybir.ActivationFunctionType.Silu)
                        nc.vector.tensor_tensor(act5[:, it, :], gs[:], pu[:],
                                                MUL)
                    for mt in range(8):
                        wt = s5w.tile([128, 8, 128], BF, tag="w2r")
                        nc.sync.dma_start(wt[:], w2rT[mt])
                        ps = s5ps.tile([128, TPC], F32, tag="pr")
                        for kt in range(8):
                            nc.tensor.matmul(ps[:], wt[:, kt, :],
                                             act5[:, kt, :],
                                             start=(kt == 0), stop=(kt == 7))
                        nc.vector.tensor_tensor(resid2_sb[:, mt, :], ps[:],
                                                resid_sb[:, mt, :], ADD)

                if stages < 6:
                    return
                # ---------- stage 6: expert (fp8 DoubleRow, all tokens) -----
                with (
                    tc.tile_pool(name="s6n", bufs=1) as s6n,
                    tc.tile_pool(name="s6w", bufs=3) as s6w,
                    tc.tile_pool(name="s6act", bufs=1) as s6act,
                    tc.tile_pool(name="s6ps", bufs=2, space="PSUM") as s6ps,
                    tc.tile_pool(name="s6tmp", bufs=3) as s6tmp,
                ):
                    nall = s6n.tile([128, 8, T], F8, tag="nall")
                    for kt in range(8):
                        nc.sync.dma_start(
                            nall[:, kt, :]
                            .rearrange("p (b t) -> p b t", b=NCORES),
                            ag_all[:, kt * 128:(kt + 1) * 128, :]
                            .rearrange("b p t -> p b t"))
                    # route row for my expert: onehot^T @ routes_all
                    routes_all = s6n.tile([E, 8, TPC], BF, tag="routes_all")
                    nc.sync.dma_start(routes_all[:],
                                      rag_all[:].rearrange("b e t -> e b t"))
                    oh_sb = s6n.tile([E, 1], BF, tag="oh")
                    nc.sync.dma_start(oh_sb[:], onehot[:])
                    route_sb = s6n.tile([1, T], F32, tag="route_sb")
                    for ch in range(4):
                        pr = s6ps.tile([1, 512], F32, tag="prow", bufs=1)
                        nc.tensor.matmul(pr[:], oh_sb[:],
                                         routes_all[:, 2 * ch:2 * ch + 2, :],
                                         start=True, stop=True)
                        # fold the fp8 descale into the route row
                        nc.vector.tensor_scalar_mul(
                            route_sb[0:1, ch * 512:(ch + 1) * 512], pr[:],
                            DESCALE)
                    routeb = s6n.tile([128, T], F32, tag="routeb")
                    nc.gpsimd.partition_broadcast(routeb[:], route_sb[0:1, :])

                    act6 = s6act.tile([128, 16, T], F8, tag="act6")
                    for it in range(16):
                        wg = s6w.tile([128, 8, 128], F8, tag="wsg")
                        nc.sync.dma_start(wg[:], wsT[it])
                        wu = s6w.tile([128, 8, 128], F8, tag="wsu")
                        nc.sync.dma_start(wu[:], wsT[16 + it])
                        for ch in range(4):
                            sl = slice(ch * 512, (ch + 1) * 512)
                            pg = s6ps.tile([128, 512], F32, tag="epg")
                            for kk in range(4):
                                nc.tensor.matmul(
                                    pg[:], wg[:, 2 * kk:2 * kk + 2, :],
                                    nall[:, 2 * kk:2 * kk + 2, sl],
                                    start=(kk == 0), stop=(kk == 3),
                                    perf_mode=DR)
                            pu = s6ps.tile([128, 512], F32, tag="epu")
                            for kk in range(4):
                                nc.tensor.matmul(
                                    pu[:], wu[:, 2 * kk:2 * kk + 2, :],
                                    nall[:, 2 * kk:2 * kk + 2, sl],
                                    start=(kk == 0), stop=(kk == 3),
                                    perf_mode=DR)
                            gs = s6tmp.tile([128, 512], BF, tag="egsil")
                            nc.scalar.activation(
                                gs[:], pg[:],
                                mybir.ActivationFunctionType.Silu,
                                scale=1.0 / SG)
                            nc.vector.tensor_tensor(act6[:, it, sl], gs[:],
                                                    pu[:], MUL)
                    for half in range(2):
                        for mt in range(half * 4, half * 4 + 4):
                            wt = s6w.tile([128, 16, 128], F8, tag="w2s")
                            nc.sync.dma_start(wt[:], w2sT[mt])
                            for ch in range(4):
                                sl = slice(ch * 512, (ch + 1) * 512)
                                ps = s6ps.tile([128, 512], F32, tag="eo")
                                for kk in range(8):
                                    nc.tensor.matmul(
                                        ps[:], wt[:, 2 * kk:2 * kk + 2, :],
                                        act6[:, 2 * kk:2 * kk + 2, sl],
                                        start=(kk == 0), stop=(kk == 7),
                                        perf_mode=DR)
                                eo = s6tmp.tile([128, 512], BF, tag="eocast")
                                nc.vector.tensor_tensor(eo[:], ps[:],
                                                        routeb[:, sl], MUL)
                                mtl = mt - half * 4
                                for jj in range(2):
                                    nc.sync.dma_start(
                                        rs_in[half][2 * ch + jj,
                                                    mtl * 128:(mtl + 1) * 128,
                                                    :],
                                        eo[:, jj * TPC:(jj + 1) * TPC])
                        nc.gpsimd.collective_compute(
                            "ReduceScatter", mybir.AluOpType.add,
                            replica_groups=[list(range(NCORES))],
                            ins=[rs_in[half][:].opt()],
                            outs=[rs_out[half][:].opt()],
                        )

                if stages < 7:
                    return
                # ---------- stage 7: final add ----------
                with tc.tile_pool(name="s7", bufs=2) as s7:
                    moe_sb = s7.tile([128, 8, TPC], BF, tag="moe_sb")
                    for half in range(2):
                        nc.sync.dma_start(
                            moe_sb[:, half * 4:half * 4 + 4, :],
                            rs_out[half][:]
                            .rearrange("(o p) t -> p o t", p=128))
                    out_sb = s7.tile([128, 8, TPC], F32, tag="out_sb")
                    for o in range(8):
                        nc.vector.tensor_tensor(out_sb[:, o, :],
                                                resid2_sb[:, o, :],
                                                moe_sb[:, o, :], ADD)
                    nc.sync.dma_start(_r3(y_dst), out_sb[:])

            # chain reps through DRAM scratch: rep r reads y of rep r-1
            y_chain = [dram.tile([H, TPC], F32, tag=f"ychain{r}",
                                 name=f"ychain{r}")
                       for r in range(reps - 1)]
            for r in range(reps):
                x_src = (xT[:] if r == 0 else
                         _r3(y_chain[r - 1][:]).rearrange('p o t -> p o t'))
                y_dst = yT[:] if r == reps - 1 else y_chain[r][:]
                emit_body(x_src, y_dst)

    nc.compile()
    return nc


def prep_inputs(positions, hidden_states, input_ln_w, post_ln_w, residual_ln_w,
                qkv_w, o_w, gate_w, ws, w2s, res_w13, res_w2):
    """Host-side fold/cast/slice -> per-core in_maps."""
    fp8 = ml_dtypes.float8_e4m3
    positions = np.asarray(positions)
    hidden_states = np.asarray(hidden_states, dtype=np.float32)

    # fold ln weights + q scale + fp8 pre-scales
    qkv_f = np.asarray(qkv_w, np.float32) * np.asarray(input_ln_w, np.float32)[None, :]
    qkv_f[:NH * HD] *= HD ** -0.5
    w13_f = np.asarray(res_w13, np.float32) * np.asarray(residual_ln_w, np.float32)[None, :]
    gate_f = np.asarray(gate_w, np.float32) * np.asarray(post_ln_w, np.float32)[None, :]
    ws_f = np.asarray(ws, np.float32) * np.asarray(post_ln_w, np.float32)[None, None, :]
    ws_f[:, :I] *= SG
    ws_f[:, I:] *= SU
    w2s_f = np.asarray(w2s, np.float32) * SW2

    def f8(x):
        return np.clip(x, -224.0, 224.0).astype(fp8)

    def tiled(wT):
        # [K, M] -> [M//128, 128(p of K), K//128, 128(m)] contiguous lhsT tiles
        K_, M_ = wT.shape
        return np.ascontiguousarray(
            wT.reshape(K_ // 128, 128, M_ // 128, 128).transpose(2, 1, 0, 3))

    wqkT = tiled(qkv_f[:QKF].T.astype(bf16))              # [10,128,8,128]
    wvT = np.ascontiguousarray(                           # [128, 8, VF]
        qkv_f[QKF:].T.astype(bf16).reshape(8, 128, VF).transpose(1, 0, 2))
    woT = tiled(np.asarray(o_w, np.float32).T.astype(bf16))
    w13T = tiled(w13_f.T.astype(bf16))
    w2rT = tiled(np.asarray(res_w2, np.float32).T.astype(bf16))
    gT = np.ascontiguousarray(gate_f.T, dtype=np.float32)  # [H, E] fp32

    # rope tables (match reference f32 math)
    pos_f = positions.astype(np.float32)
    inv_freq = (1.0 / (ROPE_BASE ** (np.arange(HALF, dtype=np.float32) / HALF))
                ).astype(np.float32)
    freqs = pos_f[:, None] * inv_freq[None, :]            # [T, 32]
    cosT = np.cos(freqs).T.astype(np.float32)             # [32, T]
    sinT = np.sin(freqs).T.astype(np.float32)

    kidx = np.arange(T)[:, None]
    in_maps = []
    for c in range(NCORES):
        sl = slice(c * TPC, (c + 1) * TPC)
        qidx = np.arange(c * TPC, (c + 1) * TPC)[None, :]
        mask = (kidx <= qidx).astype(np.float32).astype(bf16)  # [T, TPC] 0/1
        onehot = np.zeros((E, 1), bf16)
        onehot[c, 0] = 1.0
        in_maps.append({
            "xT": np.ascontiguousarray(
                hidden_states[sl].T.reshape(8, 128, TPC).transpose(1, 0, 2)),
            "cosr": np.ascontiguousarray(np.tile(cosT[:, sl], (4, 1))),
            "sinr": np.ascontiguousarray(np.tile(sinT[:, sl], (4, 1))),
            "maskT": np.ascontiguousarray(
                mask.reshape(16, 128, TPC).transpose(1, 0, 2)),
            "wqkT": np.ascontiguousarray(wqkT),
            "wvT": np.ascontiguousarray(wvT),
            "woT": np.ascontiguousarray(woT),
            "w13T": np.ascontiguousarray(w13T),
            "w2rT": np.ascontiguousarray(w2rT),
            "gT": gT,
            "wsT": tiled(f8(ws_f[c].T)),
            "w2sT": tiled(f8(w2s_f[c].T)),
            "onehot": onehot,
        })
    return in_maps


_NC_CACHE = None


def get_nc():
    global _NC_CACHE
    if _NC_CACHE is None:
        _NC_CACHE = build_nc()
    return _NC_CACHE


def kernel(**inputs):
    nc = get_nc()
    in_maps = prep_inputs(**inputs)
    res = run_bass_kernel_spmd(nc, in_maps, core_ids=list(range(NCORES)))
    out = np.concatenate(
        [res.results[c]["yT"].T for c in range(NCORES)], axis=0)
    return out.astype(np.float32)


# revision 27
# speedup vs baseline: 1.0090x; 1.0090x over previous
"""Trainium2 Bass kernel for nn_ArcticDecoderLayer (8-core SPMD).

Sharding:
  - Attention + parallel-residual MLP: token-parallel (core c owns tokens
    [256c, 256c+256)). K/V are computed locally (with RoPE) and AllGathered.
  - MoE: expert-parallel, computed densely exactly like the reference
    (every expert sees every token, outputs weighted by top-2 routes).
    Core c holds expert c; partial outputs are ReduceScattered back to the
    token owner (split in two feature halves to overlap with compute).

Layout convention: activations are stored feature-major ("transposed",
[features on partitions, tokens on free axis]) so every matmul contracts
over the partition axis. RMS-norm reductions over features use a
matmul-with-ones trick; per-token scales are applied via
gpsimd.partition_broadcast + DVE multiplies.

Precision: matmuls in bf16 except the gate (fp32, so top-2 routing matches
the reference) and the expert FFN (fp8 e4m3 with DoubleRow; weights are
pre-scaled on the host -- g rows x64, u rows x8, w2 x64 -- and the 1/512
descale is folded into the route row).

Host-side preprocessing folds input_ln_w / residual_ln_w / post_ln_w and the
attention 1/sqrt(hd) scale into the consumer weight matrices, precomputes
RoPE cos/sin tables from `positions`, and builds the additive causal mask.
"""

import numpy as np
import ml_dtypes

import concourse.bass as bass
import concourse.mybir as mybir
import concourse.tile as tile
from concourse import bacc
from concourse.bass_utils import run_bass_kernel_spmd
from concourse.masks import make_identity

# Problem constants (hardcoded per contract).
T, H, NH, NKV, HD = 2048, 1024, 16, 4, 64
I, E = 2048, 8
EPS = 1e-5
ROPE_BASE = 10000.0
NCORES = 8
TPC = T // NCORES          # 256 tokens per core
QKF = (NH + NKV) * HD      # 1280 q+k features
VF = NKV * HD              # 256 v features
HALF = HD // 2             # 32
NEG = -1.0e9
# fp8 pre-scales: g_psum = SG*g (silu applies 1/SG); act = silu(g)*SU*u;
# eo_psum = SW2*SU*eo_true -> descale 1/(SU*SW2) folded into the route row.
SG, SU, SW2 = 64.0, 8.0, 64.0
DESCALE = 1.0 / (SU * SW2)

BF = mybir.dt.bfloat16
F32 = mybir.dt.float32
F8 = mybir.dt.float8e4

bf16 = ml_dtypes.bfloat16


def _r3(ap):
    """[K, M] dram AP -> [128, K//128, M] partition-tiled view."""
    return ap.rearrange("(o p) m -> p o m", p=128)


def build_nc(reps=1, stages=99):
    nc = bacc.Bacc("TRN2", target_bir_lowering=False, debug=False,
                   num_devices=NCORES)

    # ---- per-core external inputs ----
    xT = nc.dram_tensor("xT", [128, 8, TPC], F32, kind="ExternalInput")
    cosr = nc.dram_tensor("cosr", [128, TPC], F32, kind="ExternalInput")
    sinr = nc.dram_tensor("sinr", [128, TPC], F32, kind="ExternalInput")
    maskT = nc.dram_tensor("maskT", [128, 16, TPC], BF, kind="ExternalInput")
    wqkT = nc.dram_tensor("wqkT", [10, 128, 8, 128], BF, kind="ExternalInput")
    wvT = nc.dram_tensor("wvT", [128, 8, VF], BF, kind="ExternalInput")
    woT = nc.dram_tensor("woT", [8, 128, 8, 128], BF, kind="ExternalInput")
    w13T = nc.dram_tensor("w13T", [16, 128, 8, 128], BF, kind="ExternalInput")
    w2rT = nc.dram_tensor("w2rT", [8, 128, 8, 128], BF, kind="ExternalInput")
    gT = nc.dram_tensor("gT", [H, E], F32, kind="ExternalInput")
    wsT = nc.dram_tensor("wsT", [32, 128, 8, 128], F8, kind="ExternalInput")
    w2sT = nc.dram_tensor("w2sT", [8, 128, 16, 128], F8, kind="ExternalInput")
    onehot = nc.dram_tensor("onehot", [E, 1], BF, kind="ExternalInput")
    yT = nc.dram_tensor("yT", [H, TPC], F32, kind="ExternalOutput")

    AX = mybir.AxisListType.X
    MUL = mybir.AluOpType.mult
    ADD = mybir.AluOpType.add
    SUB = mybir.AluOpType.subtract
    DR = mybir.MatmulPerfMode.DoubleRow

    with tile.TileContext(nc) as tc:
        with (
            tc.tile_pool(name="dram", bufs=1, space="DRAM") as dram,
            tc.tile_pool(name="const", bufs=1) as const,
            tc.tile_pool(name="persist", bufs=1) as persist,
        ):
            ones_col = const.tile([128, 1], F32, tag="ones")
            nc.vector.memset(ones_col[:], 1.0)
            eps_sb = const.tile([128, 1], F32, tag="eps")
            nc.vector.memset(eps_sb[:], EPS)
            ident = const.tile([128, 128], F32, tag="ident")
            make_identity(nc, ident[:])

            cos_sb = persist.tile([128, TPC], F32, tag="cos_sb")
            sin_sb = persist.tile([128, TPC], F32, tag="sin_sb")
            nc.sync.dma_start(cos_sb[:], cosr[:])
            nc.sync.dma_start(sin_sb[:], sinr[:])

            def emit_body(x_src, y_dst):
                # DRAM scratch for collectives (tags shared across reps ->
                # slot reuse serializes reps, which is what we want)
                kv_in = dram.tile([2, VF, TPC], BF, tag="kv_in")
                kv_all = dram.tile([NCORES, 2, VF, TPC], BF, tag="kv_all",
                                   addr_space="Shared")
                nag = dram.tile([H, TPC], F8, tag="nag")
                ag_all = dram.tile([NCORES, H, TPC], F8, tag="ag_all",
                                   addr_space="Shared")
                rag = dram.tile([E, TPC], BF, tag="rag")
                rag_all = dram.tile([NCORES, E, TPC], BF, tag="rag_all",
                                    addr_space="Shared")
                rs_in = [dram.tile([NCORES, H // 2, TPC], BF, tag=f"rs_in{i}",
                                   name=f"rs_in{i}")
                         for i in range(2)]
                rs_out = [dram.tile([H // 2, TPC], BF, tag=f"rs_out{i}",
                                    name=f"rs_out{i}")
                          for i in range(2)]

                # persistent sbuf (per body)
                x_sb = persist.tile([128, 8, TPC], F32, tag="x_sb")
                xn_sb = persist.tile([128, 8, TPC], BF, tag="xn_sb")
                resid_sb = persist.tile([128, 8, TPC], F32, tag="resid_sb")
                n_sb = persist.tile([128, 8, TPC], BF, tag="n_sb")
                resid2_sb = persist.tile([128, 8, TPC], F32, tag="resid2_sb")

                nc.sync.dma_start(x_sb[:], x_src)


                # ---------- helpers ----------
                def rms_scale(src_tiles, out_tiles, tmp_pool, psum_pool,
                              inv_out=None):
                    """out = src * rsqrt(mean_f(src^2)+eps), feature-major."""
                    ps = psum_pool.tile([1, TPC], F32, tag="ssq")
                    for o in range(8):
                        sq = tmp_pool.tile([128, TPC], F32, tag="sq")
                        nc.vector.tensor_tensor(sq[:], src_tiles[:, o, :],
                                                src_tiles[:, o, :], MUL)
                        nc.tensor.matmul(ps[:], ones_col[:], sq[:],
                                         start=(o == 0), stop=(o == 7))
                    sq2 = tmp_pool.tile([1, TPC], F32, tag="sqv")
                    nc.scalar.activation(sq2[:], ps[:],
                                         mybir.ActivationFunctionType.Sqrt,
                                         bias=eps_sb[0:1, :], scale=1.0 / H)
                    inv = (inv_out if inv_out is not None
                           else tmp_pool.tile([1, TPC], F32, tag="inv"))
                    nc.vector.reciprocal(inv[0:1, :], sq2[:])
                    invb = tmp_pool.tile([128, TPC], F32, tag="invb")
                    nc.gpsimd.partition_broadcast(invb[:], inv[0:1, :])
                    for o in range(8):
                        nc.vector.tensor_tensor(out_tiles[:, o, :],
                                                src_tiles[:, o, :], invb[:],
                                                MUL)

                def rope_tile(psum_ap, outs, tmp_pool):
                    """NeoX rope on a [128, TPC] psum tile holding 2 heads.

                    outs = (apA, apB): [64, TPC] destinations at partition
                    base 0 for heads at psum partitions [0:64] / [64:128]."""
                    for hh, out_ap in enumerate(outs):
                        b = 64 * hh
                        x1 = psum_ap[b:b + 32, :]
                        x2 = psum_ap[b + 32:b + 64, :]
                        ta = tmp_pool.tile([64, TPC], F32, tag="rope_a")
                        tb = tmp_pool.tile([64, TPC], F32, tag="rope_b")
                        nc.vector.tensor_tensor(ta[0:32, :], x1,
                                                cos_sb[b:b + 32, :], MUL)
                        nc.vector.tensor_tensor(tb[0:32, :], x2,
                                                sin_sb[b:b + 32, :], MUL)
                        nc.vector.tensor_tensor(out_ap[0:32, :],
                                                ta[0:32, :], tb[0:32, :], SUB)
                        nc.vector.tensor_tensor(ta[32:64, :], x2,
                                                cos_sb[b + 32:b + 64, :], MUL)
                        nc.vector.tensor_tensor(tb[32:64, :], x1,
                                                sin_sb[b + 32:b + 64, :], MUL)
                        nc.vector.tensor_tensor(out_ap[32:64, :],
                                                ta[32:64, :], tb[32:64, :],
                                                ADD)

                # ---------- stage 0: input rms norm ----------
                with (
                    tc.tile_pool(name="s0tmp", bufs=3) as s0tmp,
                    tc.tile_pool(name="s0ps", bufs=1, space="PSUM") as s0ps,
                ):
                    rms_scale(x_sb, xn_sb, s0tmp, s0ps)

                if stages < 1:
                    return
                # ---------- stage 1: qkv + rope + kv allgather ----------
                with (
                    tc.tile_pool(name="s1w", bufs=3) as s1w,
                    tc.tile_pool(name="s1ps", bufs=3, space="PSUM") as s1ps,
                    tc.tile_pool(name="s1tmp", bufs=2) as s1tmp,
                ):
                    q_sb = persist.tile([64, 16, TPC], BF, tag="q_sb")

                    def qk_tile(mt):
                        wt = s1w.tile([128, 8, 128], BF, tag="wqk", name="wt")
                        nc.sync.dma_start(wt[:], wqkT[mt])
                        ps = s1ps.tile([128, TPC], F32, tag="qk", name="ps")
                        for kt in range(8):
                            nc.tensor.matmul(ps[:], wt[:, kt, :],
                                             xn_sb[:, kt, :],
                                             start=(kt == 0), stop=(kt == 7))
                        if mt < 8:
                            rope_tile(ps[:], (q_sb[:, 2 * mt, :],
                                              q_sb[:, 2 * mt + 1, :]), s1tmp)
                        else:
                            kr = s1tmp.tile([64, 2, TPC], BF, tag="krope",
                                            name="kr")
                            rope_tile(ps[:], (kr[:, 0, :], kr[:, 1, :]), s1tmp)
                            nc.sync.dma_start(
                                kv_in[0, (mt - 8) * 128:(mt - 7) * 128, :]
                                .rearrange("(hh d) t -> d hh t", hh=2), kr[:])

                    # k and v first so the AllGather overlaps the q tiles
                    qk_tile(8)
                    qk_tile(9)
                    wvt = s1w.tile([128, 8, VF], BF, tag="wv")
                    nc.sync.dma_start(wvt[:], wvT[:])
                    for qt in range(2):
                        ps = s1ps.tile([128, VF], F32, tag="v", bufs=2)
                        for kt in range(8):
                            nc.tensor.matmul(
                                ps[:], xn_sb[:, kt, qt * 128:(qt + 1) * 128],
                                wvt[:, kt, :], start=(kt == 0), stop=(kt == 7))
                        vv = s1tmp.tile([128, VF], BF, tag="vcast")
                        nc.vector.tensor_copy(vv[:], ps[:])
                        nc.sync.dma_start(kv_in[1, qt * 128:(qt + 1) * 128, :],
                                          vv[:])
                    nc.gpsimd.collective_compute(
                        "AllGather", mybir.AluOpType.bypass,
                        replica_groups=[list(range(NCORES))],
                        ins=[kv_in[:].opt()], outs=[kv_all[:].opt()],
                    )
                    for mt in range(8):
                        qk_tile(mt)

                if stages < 2:
                    return
                # preload expert fp8 weights; DMAs overlap attention
                ws_sb = persist.tile([128, 8, 2 * I], F8, tag="ws_sb")
                for it in range(32):
                    nc.sync.dma_start(ws_sb[:, :, it * 128:(it + 1) * 128],
                                      wsT[it])
                w2s_sb = persist.tile([128, 16, H], F8, tag="w2s_sb")
                for mt in range(8):
                    nc.sync.dma_start(w2s_sb[:, :, mt * 128:(mt + 1) * 128],
                                      w2sT[mt])

                # ---------- stage 2: attention ----------
                with (
                    tc.tile_pool(name="s2kv", bufs=1) as s2kv,
                    tc.tile_pool(name="s2probs", bufs=2) as s2probs,
                    tc.tile_pool(name="s2ps", bufs=2, space="PSUM") as s2ps,
                    tc.tile_pool(name="s2pa", bufs=3, space="PSUM") as s2pa,
                    tc.tile_pool(name="s2tmp", bufs=3) as s2tmp,
                ):
                    attn_sb = persist.tile([128, 8, TPC], BF, tag="attn_sb")
                    # K feature-major at base 0: [64 d, kv head, tok]
                    k_sb = s2kv.tile([64, NKV, T], BF, tag="k_sb")
                    for kh in range(NKV):
                        nc.sync.dma_start(
                            k_sb[:, kh, :]
                            .rearrange("d (b t) -> d b t", b=NCORES),
                            kv_all[:, 0, kh * 64:(kh + 1) * 64, :]
                            .rearrange("b d t -> d b t"))
                    # V token-major + ones column: [128 tok, kt, kh, 65]
                    v_sb2 = s2kv.tile([128, 16, NKV, 65], BF, tag="v_sb2")
                    for kt in range(16):
                        b, rr = kt // 2, (kt % 2) * 128
                        nc.sync.dma_start(
                            v_sb2[:, kt, :, 0:64],
                            kv_all[b, 1, rr:rr + 128, :]
                            .rearrange("t (kh d) -> t kh d", d=64))
                    nc.vector.memset(v_sb2[:, :, :, 64:65], 1.0)
                    # mask
                    mask_sb = s2kv.tile([128, 16, TPC], BF, tag="mask_sb")
                    nc.sync.dma_start(mask_sb[:], maskT[:])

                    for h in range(NH):
                        kh = h // NKV
                        q_rhs = q_sb[:, h, :]
                        probs = s2probs.tile([128, 16, TPC], BF, tag="probs")
                        for ktg in range(4):
                            ps = s2ps.tile([128, 4, TPC], F32, tag="sc")
                            for j in range(4):
                                kt = 4 * ktg + j
                                nc.tensor.matmul(
                                    ps[:, j, :],
                                    k_sb[:, kh, kt * 128:(kt + 1) * 128],
                                    q_rhs, start=True, stop=True)
                            nc.scalar.activation(
                                probs[:, 4 * ktg:4 * ktg + 4, :], ps[:],
                                mybir.ActivationFunctionType.Exp)
                            # causal mask is multiplicative 0/1 in bf16:
                            # exp(s + m) == exp(s) * m01
                            nc.vector.tensor_tensor(
                                probs[:, 4 * ktg:4 * ktg + 4, :],
                                probs[:, 4 * ktg:4 * ktg + 4, :],
                                mask_sb[:, 4 * ktg:4 * ktg + 4, :], MUL)
                        pa = s2pa.tile([128, TPC], F32, tag="pattn")
                        for kt in range(16):
                            nc.tensor.matmul(pa[0:65, :], v_sb2[:, kt, kh, :],
                                             probs[:, kt, :],
                                             start=(kt == 0), stop=(kt == 15))
                        rec = s2tmp.tile([1, TPC], F32, tag="rec")
                        nc.vector.reciprocal(rec[:], pa[64:65, :])
                        recb = s2tmp.tile([64, TPC], F32, tag="recb")
                        nc.gpsimd.partition_broadcast(recb[:], rec[0:1, :])
                        nc.vector.tensor_tensor(
                            attn_sb[(h % 2) * 64:(h % 2) * 64 + 64, h // 2, :],
                            pa[0:64, :], recb[:], MUL)

                if stages < 3:
                    return
                # ---------- stage 3: o-proj + residual ----------
                with (
                    tc.tile_pool(name="s3w", bufs=3) as s3w,
                    tc.tile_pool(name="s3ps", bufs=4, space="PSUM") as s3ps,
                ):
                    for mt in range(8):
                        wt = s3w.tile([128, 8, 128], BF, tag="wo")
                        nc.sync.dma_start(wt[:], woT[mt])
                        ps = s3ps.tile([128, TPC], F32, tag="o")
                        for kt in range(8):
                            nc.tensor.matmul(ps[:], wt[:, kt, :],
                                             attn_sb[:, kt, :],
                                             start=(kt == 0), stop=(kt == 7))
                        nc.vector.tensor_tensor(resid_sb[:, mt, :], ps[:],
                                                x_sb[:, mt, :], ADD)

                if stages < 4:
                    return
                # ------- stage 4: post norm + fp32 gate + routes + AG -------
                with (
                    tc.tile_pool(name="s4tmp", bufs=3) as s4tmp,
                    tc.tile_pool(name="s4ps", bufs=2, space="PSUM") as s4ps,
                ):
                    inv2 = s4tmp.tile([1, TPC], F32, tag="inv2", bufs=1)
                    rms_scale(resid_sb, n_sb, s4tmp, s4ps, inv_out=inv2)
                    nf8 = s4tmp.tile([128, 8, TPC], F8, tag="nf8", bufs=1)
                    for o in range(8):
                        nc.vector.tensor_copy(nf8[:, o, :], n_sb[:, o, :])
                    nc.sync.dma_start(
                        nag[:].rearrange("(o p) t -> p o t", p=128), nf8[:])
                    nc.gpsimd.collective_compute(
                        "AllGather", mybir.AluOpType.bypass,
                        replica_groups=[list(range(NCORES))],
                        ins=[nag[:].opt()], outs=[ag_all[:].opt()],
                    )
                    # inv_rms token-major: [TPC(2x128), 1] via matmul transpose
                    invt = s4tmp.tile([128, 2, 1], F32, tag="invt", bufs=1)
                    for qt in range(2):
                        pst = s4ps.tile([128, 1], F32, tag="invtp")
                        nc.tensor.matmul(pst[:],
                                         inv2[0:1, qt * 128:(qt + 1) * 128],
                                         ones_col[0:1, :], start=True,
                                         stop=True)
                        nc.vector.tensor_copy(invt[:, qt, :], pst[:])
                    # gate logits fp32 token-major: [128 tok, E]
                    gt_sb = s4tmp.tile([128, 8, E], F32, tag="gt", bufs=1)
                    nc.sync.dma_start(gt_sb[:], _r3(gT[:]))
                    for qt in range(2):
                        lg = s4ps.tile([128, E], F32, tag="lg")
                        for kt in range(8):
                            nc.tensor.matmul(
                                lg[:],
                                resid_sb[:, kt, qt * 128:(qt + 1) * 128],
                                gt_sb[:, kt, :], start=(kt == 0),
                                stop=(kt == 7))
                        lgs = s4tmp.tile([128, E], F32, tag="lgs")
                        nc.scalar.mul(lgs[:], lg[:], invt[:, qt, :])
                        # top-2 softmax renorm, token-major
                        m1 = s4tmp.tile([128, 1], F32, tag="m1")
                        nc.vector.reduce_max(m1[:], lgs[:], axis=AX)
                        negm = s4tmp.tile([128, 1], F32, tag="negm")
                        nc.vector.tensor_scalar_mul(negm[:], m1[:], -1.0)
                        ex = s4tmp.tile([128, E], F32, tag="ex")
                        nc.scalar.activation(ex[:], lgs[:],
                                             mybir.ActivationFunctionType.Exp,
                                             bias=negm[:])
                        msk = s4tmp.tile([128, E], F32, tag="msk")
                        nc.vector.tensor_tensor(msk[:], lgs[:],
                                                m1[:].to_broadcast([128, E]),
                                                mybir.AluOpType.is_ge)
                        nc.vector.tensor_scalar_mul(msk[:], msk[:], NEG)
                        nc.vector.tensor_tensor(msk[:], lgs[:], msk[:], ADD)
                        m2 = s4tmp.tile([128, 1], F32, tag="m2")
                        nc.vector.reduce_max(m2[:], msk[:], axis=AX)
                        keep = s4tmp.tile([128, E], F32, tag="keep")
                        nc.vector.tensor_tensor(keep[:], lgs[:],
                                                m2[:].to_broadcast([128, E]),
                                                mybir.AluOpType.is_ge)
                        nc.vector.tensor_tensor(keep[:], keep[:], ex[:], MUL)
                        den = s4tmp.tile([128, 1], F32, tag="den")
                        nc.vector.reduce_sum(den[:], keep[:], axis=AX)
                        rden = s4tmp.tile([128, 1], F32, tag="rden")
                        nc.vector.reciprocal(rden[:], den[:])
                        routes = s4tmp.tile([128, E], F32, tag="routes")
                        nc.scalar.mul(routes[:], keep[:], rden[:])
                        # transpose to [E, 128] and ship bf16
                        pt = s4ps.tile([128, 128], F32, tag="rt")
                        nc.tensor.transpose(pt[0:E, :], routes[:], ident[:])
                        rbf = s4tmp.tile([E, 128], BF, tag="rbf")
                        nc.vector.tensor_copy(rbf[:], pt[0:E, :])
                        nc.sync.dma_start(rag[:, qt * 128:(qt + 1) * 128],
                                          rbf[:])

                nc.gpsimd.collective_compute(
                    "AllGather", mybir.AluOpType.bypass,
                    replica_groups=[list(range(NCORES))],
                    ins=[rag[:].opt()], outs=[rag_all[:].opt()],
                )

                if stages < 5:
                    return
                # -------- stage 5: parallel residual MLP (local tokens) -----
                with (
                    tc.tile_pool(name="s5w", bufs=3) as s5w,
                    tc.tile_pool(name="s5ps", bufs=2, space="PSUM") as s5ps,
                    tc.tile_pool(name="s5act", bufs=1) as s5act,
                ):
                    act5 = s5act.tile([128, 8, TPC], BF, tag="act5")
                    for it in range(8):
                        wg = s5w.tile([128, 8, 128], BF, tag="w13g")
                        nc.sync.dma_start(wg[:], w13T[it])
                        wu = s5w.tile([128, 8, 128], BF, tag="w13u")
                        nc.sync.dma_start(wu[:], w13T[8 + it])
                        pg = s5ps.tile([128, TPC], F32, tag="pg")
                        for kt in range(8):
                            nc.tensor.matmul(pg[:], wg[:, kt, :],
                                             n_sb[:, kt, :],
                                             start=(kt == 0), stop=(kt == 7))
                        pu = s5ps.tile([128, TPC], F32, tag="pu")
                        for kt in range(8):
                            nc.tensor.matmul(pu[:], wu[:, kt, :],
                                             n_sb[:, kt, :],
                                             start=(kt == 0), stop=(kt == 7))
                        gs = s5w.tile([128, TPC], BF, tag="gsil")
                        nc.scalar.activation(gs[:], pg[:],
                                             mybir.ActivationFunctionType.Silu)
                        nc.vector.tensor_tensor(act5[:, it, :], gs[:], pu[:],
                                                MUL)
                    for mt in range(8):
                        wt = s5w.tile([128, 8, 128], BF, tag="w2r")
                        nc.sync.dma_start(wt[:], w2rT[mt])
                        ps = s5ps.tile([128, TPC], F32, tag="pr")
                        for kt in range(8):
                            nc.tensor.matmul(ps[:], wt[:, kt, :],
                                             act5[:, kt, :],
                                             start=(kt == 0), stop=(kt == 7))
                        nc.vector.tensor_tensor(resid2_sb[:, mt, :], ps[:],
                                                resid_sb[:, mt, :], ADD)

                if stages < 6:
                    return
                # ---------- stage 6: expert (fp8 DoubleRow, all tokens) -----
                with (
                    tc.tile_pool(name="s6n", bufs=1) as s6n,
                    tc.tile_pool(name="s6w", bufs=3) as s6w,
                    tc.tile_pool(name="s6act", bufs=1) as s6act,
                    tc.tile_pool(name="s6ps", bufs=2, space="PSUM") as s6ps,
                    tc.tile_pool(name="s6tmp", bufs=3) as s6tmp,
                ):
                    nall = s6n.tile([128, 8, T], F8, tag="nall")
                    for kt in range(8):
                        nc.sync.dma_start(
                            nall[:, kt, :]
                            .rearrange("p (b t) -> p b t", b=NCORES),
                            ag_all[:, kt * 128:(kt + 1) * 128, :]
                            .rearrange("b p t -> p b t"))
                    # route row for my expert: onehot^T @ routes_all
                    routes_all = s6n.tile([E, 8, TPC], BF, tag="routes_all")
                    nc.sync.dma_start(routes_all[:],
                                      rag_all[:].rearrange("b e t -> e b t"))
                    oh_sb = s6n.tile([E, 1], BF, tag="oh")
                    nc.sync.dma_start(oh_sb[:], onehot[:])
                    route_sb = s6n.tile([1, T], F32, tag="route_sb")
                    for ch in range(4):
                        pr = s6ps.tile([1, 512], F32, tag="prow", bufs=1)
                        nc.tensor.matmul(pr[:], oh_sb[:],
                                         routes_all[:, 2 * ch:2 * ch + 2, :],
                                         start=True, stop=True)
                        # fold the fp8 descale into the route row
                        nc.vector.tensor_scalar_mul(
                            route_sb[0:1, ch * 512:(ch + 1) * 512], pr[:],
                            DESCALE)
                    routeb = s6n.tile([128, T], F32, tag="routeb")
                    nc.gpsimd.partition_broadcast(routeb[:], route_sb[0:1, :])

                    act6 = s6act.tile([128, 16, T], F8, tag="act6")
                    for it in range(16):
                        wg = ws_sb[:, :, it * 128:(it + 1) * 128]
                        wu = ws_sb[:, :, (16 + it) * 128:(17 + it) * 128]
                        for ch in range(4):
                            sl = slice(ch * 512, (ch + 1) * 512)
                            pg = s6ps.tile([128, 512], F32, tag="epg")
                            for kk in range(4):
                                nc.tensor.matmul(
                                    pg[:], wg[:, 2 * kk:2 * kk + 2, :],
                                    nall[:, 2 * kk:2 * kk + 2, sl],
                                    start=(kk == 0), stop=(kk == 3),
                                    perf_mode=DR)
                            pu = s6ps.tile([128, 512], F32, tag="epu")
                            for kk in range(4):
                                nc.tensor.matmul(
                                    pu[:], wu[:, 2 * kk:2 * kk + 2, :],
                                    nall[:, 2 * kk:2 * kk + 2, sl],
                                    start=(kk == 0), stop=(kk == 3),
                                    perf_mode=DR)
                            gs = s6tmp.tile([128, 512], BF, tag="egsil")
                            nc.scalar.activation(
                                gs[:], pg[:],
                                mybir.ActivationFunctionType.Silu,
                                scale=1.0 / SG)
                            nc.vector.tensor_tensor(act6[:, it, sl], gs[:],
                                                    pu[:], MUL)
                    for half in range(2):
                        for mt in range(half * 4, half * 4 + 4):
                            wt = w2s_sb[:, :, mt * 128:(mt + 1) * 128]
                            for ch in range(4):
                                sl = slice(ch * 512, (ch + 1) * 512)
                                ps = s6ps.tile([128, 512], F32, tag="eo")
                                for kk in range(8):
                                    nc.tensor.matmul(
                                        ps[:], wt[:, 2 * kk:2 * kk + 2, :],
                                        act6[:, 2 * kk:2 * kk + 2, sl],
                                        start=(kk == 0), stop=(kk == 7),
                                        perf_mode=DR)
                                eo = s6tmp.tile([128, 512], BF, tag="eocast")
                                nc.vector.tensor_tensor(eo[:], ps[:],
                                                        routeb[:, sl], MUL)
                                mtl = mt - half * 4
                                for jj in range(2):
                                    nc.sync.dma_start(
                                        rs_in[half][2 * ch + jj,
                                                    mtl * 128:(mtl + 1) * 128,
                                                    :],
                                        eo[:, jj * TPC:(jj + 1) * TPC])
                        nc.gpsimd.collective_compute(
                            "ReduceScatter", mybir.AluOpType.add,
                            replica_groups=[list(range(NCORES))],
                            ins=[rs_in[half][:].opt()],
                            outs=[rs_out[half][:].opt()],
                        )

                if stages < 7:
                    return
                # ---------- stage 7: final add ----------
                with tc.tile_pool(name="s7", bufs=2) as s7:
                    moe_sb = s7.tile([128, 8, TPC], BF, tag="moe_sb")
                    for half in range(2):
                        nc.sync.dma_start(
                            moe_sb[:, half * 4:half * 4 + 4, :],
                            rs_out[half][:]
                            .rearrange("(o p) t -> p o t", p=128))
                    out_sb = s7.tile([128, 8, TPC], F32, tag="out_sb")
                    for o in range(8):
                        nc.vector.tensor_tensor(out_sb[:, o, :],
                                                resid2_sb[:, o, :],
                                                moe_sb[:, o, :], ADD)
                    nc.sync.dma_start(_r3(y_dst), out_sb[:])

            # chain reps through DRAM scratch: rep r reads y of rep r-1
            y_chain = [dram.tile([H, TPC], F32, tag=f"ychain{r}",
                                 name=f"ychain{r}")
                       for r in range(reps - 1)]
            for r in range(reps):
                x_src = (xT[:] if r == 0 else
                         _r3(y_chain[r - 1][:]).rearrange('p o t -> p o t'))
                y_dst = yT[:] if r == reps - 1 else y_chain[r][:]
                emit_body(x_src, y_dst)

    nc.compile()
    return nc


def prep_inputs(positions, hidden_states, input_ln_w, post_ln_w, residual_ln_w,
                qkv_w, o_w, gate_w, ws, w2s, res_w13, res_w2):
    """Host-side fold/cast/slice -> per-core in_maps."""
    fp8 = ml_dtypes.float8_e4m3
    positions = np.asarray(positions)
    hidden_states = np.asarray(hidden_states, dtype=np.float32)

    # fold ln weights + q scale + fp8 pre-scales
    qkv_f = np.asarray(qkv_w, np.float32) * np.asarray(input_ln_w, np.float32)[None, :]
    qkv_f[:NH * HD] *= HD ** -0.5
    w13_f = np.asarray(res_w13, np.float32) * np.asarray(residual_ln_w, np.float32)[None, :]
    gate_f = np.asarray(gate_w, np.float32) * np.asarray(post_ln_w, np.float32)[None, :]
    ws_f = np.asarray(ws, np.float32) * np.asarray(post_ln_w, np.float32)[None, None, :]
    ws_f[:, :I] *= SG
    ws_f[:, I:] *= SU
    w2s_f = np.asarray(w2s, np.float32) * SW2

    def f8(x):
        return np.clip(x, -224.0, 224.0).astype(fp8)

    def tiled(wT):
        # [K, M] -> [M//128, 128(p of K), K//128, 128(m)] contiguous lhsT tiles
        K_, M_ = wT.shape
        return np.ascontiguousarray(
            wT.reshape(K_ // 128, 128, M_ // 128, 128).transpose(2, 1, 0, 3))

    wqkT = tiled(qkv_f[:QKF].T.astype(bf16))              # [10,128,8,128]
    wvT = np.ascontiguousarray(                           # [128, 8, VF]
        qkv_f[QKF:].T.astype(bf16).reshape(8, 128, VF).transpose(1, 0, 2))
    woT = tiled(np.asarray(o_w, np.float32).T.astype(bf16))
    w13T = tiled(w13_f.T.astype(bf16))
    w2rT = tiled(np.asarray(res_w2, np.float32).T.astype(bf16))
    gT = np.ascontiguousarray(gate_f.T, dtype=np.float32)  # [H, E] fp32

    # rope tables (match reference f32 math)
    pos_f = positions.astype(np.float32)
    inv_freq = (1.0 / (ROPE_BASE ** (np.arange(HALF, dtype=np.float32) / HALF))
                ).astype(np.float32)
    freqs = pos_f[:, None] * inv_freq[None, :]            # [T, 32]
    cosT = np.cos(freqs).T.astype(np.float32)             # [32, T]
    sinT = np.sin(freqs).T.astype(np.float32)

    kidx = np.arange(T)[:, None]
    in_maps = []
    for c in range(NCORES):
        sl = slice(c * TPC, (c + 1) * TPC)
        qidx = np.arange(c * TPC, (c + 1) * TPC)[None, :]
        mask = (kidx <= qidx).astype(np.float32).astype(bf16)  # [T, TPC] 0/1
        onehot = np.zeros((E, 1), bf16)
        onehot[c, 0] = 1.0
        in_maps.append({
            "xT": np.ascontiguousarray(
                hidden_states[sl].T.reshape(8, 128, TPC).transpose(1, 0, 2)),
            "cosr": np.ascontiguousarray(np.tile(cosT[:, sl], (4, 1))),
            "sinr": np.ascontiguousarray(np.tile(sinT[:, sl], (4, 1))),
            "maskT": np.ascontiguousarray(
                mask.reshape(16, 128, TPC).transpose(1, 0, 2)),
            "wqkT": np.ascontiguousarray(wqkT),
            "wvT": np.ascontiguousarray(wvT),
            "woT": np.ascontiguousarray(woT),
            "w13T": np.ascontiguousarray(w13T),
            "w2rT": np.ascontiguousarray(w2rT),
            "gT": gT,
            "wsT": tiled(f8(ws_f[c].T)),
            "w2sT": tiled(f8(w2s_f[c].T)),
            "onehot": onehot,
        })
    return in_maps


_NC_CACHE = None


def get_nc():
    global _NC_CACHE
    if _NC_CACHE is None:
        _NC_CACHE = build_nc()
    return _NC_CACHE


def kernel(**inputs):
    nc = get_nc()
    in_maps = prep_inputs(**inputs)
    res = run_bass_kernel_spmd(nc, in_maps, core_ids=list(range(NCORES)))
    out = np.concatenate(
        [res.results[c]["yT"].T for c in range(NCORES)], axis=0)
    return out.astype(np.float32)
